# revision 21
# baseline (speedup 1.0000x reference)
"""GCN encoder (2x GCNConv + mu/logvar heads) on 8 Trainium2 NeuronCores.

Strategy (dst-sharded message passing, PE one-hot scatter):
  - Nodes are dst-sharded across 8 cores (6250 original rows each, padded
    to 6400 = 25 "pair-tiles" of 256 dst nodes).
  - All per-edge norm factors fold into the one-hot selection matrix built
    with a single fused DVE tensor_scalar (is_equal then *norm) per 128-edge
    subtile; scatter-add happens in PSUM via PE matmuls (float32r).
  - Source features are fetched with dma_gather (SDMA descriptor gather,
    512B rows) from a node table in DRAM. int16 gather indices force the
    table to be addressed as two halves (rows < 25600 and >= 25600).
  - The per-layer dense transform commutes with the scatter, so each layer
    is: aggregate (A^T tile in PSUM) -> W matmul -> bias+act -> transpose ->
    store node-major shard. mu/logvar share one aggregation pass.
  - Between layers, shards are exchanged with a shared-output AllGather into
    a Shared-scratchpad full table which the next pass gathers from.
"""

import os
import sys
from dataclasses import dataclass

import numpy as np

if "/opt/trn_rl_repo" not in sys.path:
    sys.path.insert(0, "/opt/trn_rl_repo")

N = 50000
E = 800000
IN_C = 128
HID = 128
OUT_C = 64
M = 8  # cores


@dataclass(frozen=True)
class Cfg:
    n: int            # real nodes
    n_cores: int
    shard: int        # real nodes per core (n = shard * n_cores)
    shard_pad: int    # padded nodes per core, multiple of 256
    table_split: int  # row count of the "lo" gather table (<= 32767)

    @property
    def npad(self):
        return self.shard_pad * self.n_cores

    @property
    def n_pair(self):
        return self.shard_pad // 256


REAL_CFG = Cfg(n=N, n_cores=M, shard=6250, shard_pad=6400, table_split=25600)

ST = 128  # edges per subtile
GSUB = 8  # max subtiles (x128 idx) per dma_gather instruction
KNOCK_GATHER = bool(int(os.environ.get("KNOCK_GATHER", "0")))
KNOCK_SEL = bool(int(os.environ.get("KNOCK_SEL", "0")))
MM_SEG = int(os.environ.get("MM_SEG", "0"))  # 0 = one start/stop chain


def preprocess(x, edge_index, cfg: Cfg):
    """Sort/pad edges into the per-core static schedule + device input arrays."""
    x = np.asarray(x, np.float32)
    src = np.asarray(edge_index[0], np.int64)
    dst = np.asarray(edge_index[1], np.int64)
    n = cfg.n
    loop = np.arange(n, dtype=np.int64)
    src = np.concatenate([src, loop])
    dst = np.concatenate([dst, loop])
    deg = np.bincount(dst, minlength=n).astype(np.float32)
    dinv = 1.0 / np.sqrt(deg)
    norm = (dinv[src] * dinv[dst]).astype(np.float32)

    core = dst // cfg.shard
    dloc = dst % cfg.shard
    tile_id = dloc // 256
    dstloc = (dloc % 256).astype(np.float32)
    gsrc = (src // cfg.shard) * cfg.shard_pad + (src % cfg.shard)
    half = (gsrc >= cfg.table_split).astype(np.int64)
    gidx = np.where(half == 0, gsrc, gsrc - cfg.table_split).astype(np.int64)
    assert gidx.max() < 32768

    order = np.lexsort((gidx, half, tile_id, core))
    core, tile_id, half = core[order], tile_id[order], half[order]
    dstloc, norm, gidx = dstloc[order], norm[order], gidx[order]

    # counts[c, T, h]
    counts = np.zeros((cfg.n_cores, cfg.n_pair, 2), np.int64)
    np.add.at(counts, (core, tile_id, half), 1)
    nsub = np.ceil(counts.max(axis=0) / ST).astype(np.int64)  # [n_pair, 2]
    # every pair-tile contains real nodes -> self loops guarantee lo+hi >= 1;
    # still force at least one subtile total per tile for psum init
    for t in range(cfg.n_pair):
        if nsub[t].sum() == 0:
            nsub[t, 0] = 1
    S = int(nsub.sum())  # total subtiles per core

    # group start offsets (in subtiles), order: T0-lo, T0-hi, T1-lo, ...
    groups = []  # (tile, half, nsub, s0)
    s0 = 0
    for t in range(cfg.n_pair):
        for h in (0, 1):
            groups.append((t, h, int(nsub[t, h]), s0))
            s0 += int(nsub[t, h])

    # per-core padded edge arrays in schedule order
    idx_slab = np.zeros((cfg.n_cores, 128, S * 8), np.int16)
    dl_slab = np.zeros((cfg.n_cores, 128, S), np.float32)
    nm_slab = np.zeros((cfg.n_cores, 128, S), np.float32)

    # edges are sorted by (core, tile, half); find range per (c,t,h)
    key = (core * cfg.n_pair + tile_id) * 2 + half
    starts = np.searchsorted(key, np.arange(cfg.n_cores * cfg.n_pair * 2))
    ends = np.searchsorted(key, np.arange(cfg.n_cores * cfg.n_pair * 2) + 1)

    for c in range(cfg.n_cores):
        for (t, h, ns, g0) in groups:
            if ns == 0:
                continue
            k = (c * cfg.n_pair + t) * 2 + h
            a, b = starts[k], ends[k]
            cnt = b - a
            cap = ns * ST
            assert cnt <= cap
            gi = np.zeros(cap, np.int16)
            dl = np.zeros(cap, np.float32)
            nm = np.zeros(cap, np.float32)
            gi[:cnt] = gidx[a:b]
            dl[:cnt] = dstloc[a:b]
            nm[:cnt] = norm[a:b]
            cols = gi.reshape(ns * 8, 16).T  # [16, ns*8]
            idx_slab[c, :, g0 * 8:(g0 + ns) * 8] = np.tile(cols, (8, 1))
            dl_slab[c, :, g0:g0 + ns] = dl.reshape(ns, ST).T
            nm_slab[c, :, g0:g0 + ns] = nm.reshape(ns, ST).T

    # padded node table for x
    x_pad = np.zeros((cfg.npad, x.shape[1]), np.float32)
    gnode = (np.arange(n) // cfg.shard) * cfg.shard_pad + (np.arange(n) % cfg.shard)
    x_pad[gnode] = x

    return groups, S, idx_slab, dl_slab, nm_slab, x_pad, gnode


def build_program(cfg: Cfg, groups, S, feat_in, hid, out_c):
    from contextlib import ExitStack

    import concourse.tile as tile
    from concourse import bacc, mybir

    f32 = mybir.dt.float32
    f32r = mybir.dt.float32r
    i16 = mybir.dt.int16
    AF = mybir.ActivationFunctionType
    ALU = mybir.AluOpType

    nc = bacc.Bacc("TRN2", target_bir_lowering=False, debug=False,
                   num_devices=cfg.n_cores)

    npad, spad = cfg.npad, cfg.shard_pad
    xt = nc.dram_tensor("xt", [npad, feat_in], f32, kind="ExternalInput")
    idx_t = nc.dram_tensor("idx", [128, S * 8], i16, kind="ExternalInput")
    dl_t = nc.dram_tensor("dstloc", [128, S], f32, kind="ExternalInput")
    nm_t = nc.dram_tensor("norm", [128, S], f32, kind="ExternalInput")
    w1_t = nc.dram_tensor("W1", [feat_in, hid], f32r, kind="ExternalInput")
    w2_t = nc.dram_tensor("W2", [hid, hid], f32r, kind="ExternalInput")
    wml_t = nc.dram_tensor("Wml", [hid, 2 * out_c], f32r, kind="ExternalInput")
    b1_t = nc.dram_tensor("b1", [hid, 1], f32, kind="ExternalInput")
    b2_t = nc.dram_tensor("b2", [hid, 1], f32, kind="ExternalInput")
    bml_t = nc.dram_tensor("bml", [2 * out_c, 1], f32, kind="ExternalInput")
    iota_t = nc.dram_tensor("iota", [128, 256], f32, kind="ExternalInput")
    ident_t = nc.dram_tensor("ident", [128, 128], f32, kind="ExternalInput")

    mu_t = nc.dram_tensor("mu", [spad, out_c], f32, kind="ExternalOutput")
    lv_t = nc.dram_tensor("lv", [spad, out_c], f32, kind="ExternalOutput")

    h1s = nc.dram_tensor("h1s", [spad, hid], f32)
    h2s = nc.dram_tensor("h2s", [spad, hid], f32)
    h1f = nc.dram_tensor("h1f", [npad, hid], f32, addr_space="Shared")
    h2f = nc.dram_tensor("h2f", [npad, hid], f32, addr_space="Shared")

    max_ns = max(g[2] for g in groups)
    replica = [list(range(cfg.n_cores))]

    with tile.TileContext(nc) as tc, ExitStack() as ctx:
        cpool = ctx.enter_context(tc.tile_pool(name="consts", bufs=1))
        gpool = ctx.enter_context(tc.tile_pool(name="gather", bufs=4))
        spool = ctx.enter_context(tc.tile_pool(name="sel", bufs=6))
        wpool = ctx.enter_context(tc.tile_pool(name="work", bufs=3))
        ppool = ctx.enter_context(tc.tile_pool(name="psum", bufs=2, space="PSUM"))
        ipool = ctx.enter_context(tc.tile_pool(name="iotap", bufs=1, space="PSUM"))

        def load_const(t, shape, dtype=f32):
            tl = cpool.tile(shape, dtype, tag=t.name)
            nc.sync.dma_start(tl[:], t.ap())
            return tl

        idx_sb = load_const(idx_t, [128, S * 8], i16)
        dl_sb = load_const(dl_t, [128, S])
        nm_sb = load_const(nm_t, [128, S])
        w1_sb = load_const(w1_t, [feat_in, hid], f32r)
        w2_sb = load_const(w2_t, [hid, hid], f32r)
        wml_sb = load_const(wml_t, [hid, 2 * out_c], f32r)
        b1_sb = load_const(b1_t, [hid, 1])
        b2_sb = load_const(b2_t, [hid, 1])
        bml_sb = load_const(bml_t, [2 * out_c, 1])
        iota_sb = load_const(iota_t, [128, 256])
        ident_sb = load_const(ident_t, [128, 128])
        # iota lives in PSUM: a DVE tensor_scalar with two SBUF tensor
        # operands enters 2-port perf mode, which locks GpSimd out of SBUF
        # while it is emitting SWDGE gather descriptors (hangs the device).
        iota_ps = ipool.tile([128, 256], f32, tag="iota_ps")
        nc.vector.tensor_copy(iota_ps[:], iota_sb[:])

        passes = [
            (xt, w1_sb, b1_sb, h1s, True),
            (h1f, w2_sb, b2_sb, h2s, True),
            (h2f, wml_sb, bml_sb, None, False),
        ]

        for pi, (table, w_sb, b_sb, shard_out, relu) in enumerate(passes):
            t_lo = table.ap()[:cfg.table_split, :]
            t_hi = table.ap()[cfg.table_split:, :]
            for t in range(cfg.n_pair):
                tgroups = [g for g in groups if g[0] == t and g[2] > 0]
                n_mm = sum(g[2] for g in tgroups)
                at_psum = ppool.tile([128, 256], f32, tag="at")
                mm = 0
                for (_, h, ns, s0) in tgroups:
                    src_ap = t_lo if h == 0 else t_hi
                    for c0 in range(0, ns, GSUB):
                        cn = min(GSUB, ns - c0)
                        g = gpool.tile([128, GSUB, feat_in], f32r, tag="g")
                        if KNOCK_GATHER:
                            nc.vector.memset(g[:, :cn, :].bitcast(f32), 0.25)
                        else:
                            nc.gpsimd.dma_gather(
                                g[:, :cn, :], src_ap.bitcast(f32r),
                                idx_sb[:, (s0 + c0) * 8:(s0 + c0 + cn) * 8],
                                num_idxs=cn * ST, num_idxs_reg=cn * ST,
                                elem_size=feat_in,
                            )
                        for k in range(cn):
                            s = s0 + c0 + k
                            sel = spool.tile([128, 256], f32r, tag="sel")
                            if KNOCK_SEL:
                                nc.vector.memset(sel[:].bitcast(f32), 0.001)
                            else:
                                nc.vector.tensor_scalar(
                                    sel[:], iota_ps[:],
                                    dl_sb[:, s:s + 1], nm_sb[:, s:s + 1],
                                    op0=ALU.is_equal, op1=ALU.mult,
                                )
                            if MM_SEG == 0:
                                start, stop = (mm == 0), (mm == n_mm - 1)
                            else:
                                start = mm == 0
                                stop = (mm % MM_SEG == MM_SEG - 1) or (mm == n_mm - 1)
                            nc.tensor.matmul(
                                at_psum[:],
                                lhsT=g[:, k, :],
                                rhs=sel[:],
                                start=start, stop=stop,
                            )
                            mm += 1
                at_sb = wpool.tile([128, 256], f32r, tag="at_sb")
                nc.vector.tensor_copy(at_sb[:], at_psum[:])
                ht_psum = ppool.tile([128, 256], f32, tag="ht")
                nc.tensor.matmul(ht_psum[:], lhsT=w_sb[:],
                                 rhs=at_sb[:],
                                 start=True, stop=True)
                h_sb = wpool.tile([128, 256], f32, tag="h_sb")
                nc.scalar.activation(h_sb[:], ht_psum[:],
                                     AF.Relu if relu else AF.Identity,
                                     bias=b_sb[:, :1])
                for j in (0, 1):
                    tp = ppool.tile([128, 128], f32, tag="tp")
                    nc.tensor.transpose(tp[:], h_sb[:, j * 128:(j + 1) * 128],
                                        ident_sb[:])
                    ts = wpool.tile([128, 128], f32, tag="ts")
                    nc.vector.tensor_copy(ts[:], tp[:])
                    r0 = t * 256 + j * 128
                    if shard_out is not None:
                        nc.sync.dma_start(out=shard_out.ap()[r0:r0 + 128, :],
                                          in_=ts[:])
                    else:
                        nc.sync.dma_start(out=mu_t.ap()[r0:r0 + 128, :],
                                          in_=ts[:, 0:out_c])
                        nc.sync.dma_start(out=lv_t.ap()[r0:r0 + 128, :],
                                          in_=ts[:, out_c:2 * out_c])
            if pi == 0:
                nc.gpsimd.collective_compute(
                    "AllGather", mybir.AluOpType.bypass,
                    replica_groups=replica,
                    ins=[h1s.ap()], outs=[h1f.ap()])
            elif pi == 1:
                nc.gpsimd.collective_compute(
                    "AllGather", mybir.AluOpType.bypass,
                    replica_groups=replica,
                    ins=[h2s.ap()], outs=[h2f.ap()])
    nc.compile()
    return nc


def _run(inputs, cfg=REAL_CFG, trace=False):
    x = np.asarray(inputs["x"], np.float32)
    edge_index = np.asarray(inputs["edge_index"])
    W1 = np.asarray(inputs["W1"], np.float32)
    b1 = np.asarray(inputs["b1"], np.float32)
    W2 = np.asarray(inputs["W2"], np.float32)
    b2 = np.asarray(inputs["b2"], np.float32)
    W_mu = np.asarray(inputs["W_mu"], np.float32)
    b_mu = np.asarray(inputs["b_mu"], np.float32)
    W_lv = np.asarray(inputs["W_lv"], np.float32)
    b_lv = np.asarray(inputs["b_lv"], np.float32)

    feat_in = x.shape[1]
    hid = W1.shape[1]
    out_c = W_mu.shape[1]

    groups, S, idx_slab, dl_slab, nm_slab, x_pad, gnode = preprocess(
        x, edge_index, cfg)
    nc = build_program(cfg, groups, S, feat_in, hid, out_c)

    wml = np.concatenate([W_mu, W_lv], axis=1)
    bml = np.concatenate([b_mu, b_lv])[:, None].astype(np.float32)
    iota = np.tile(np.arange(256, dtype=np.float32), (128, 1))
    ident = np.eye(128, dtype=np.float32)

    in_maps = []
    for c in range(cfg.n_cores):
        in_maps.append({
            "xt": x_pad,
            "idx": idx_slab[c],
            "dstloc": dl_slab[c],
            "norm": nm_slab[c],
            "W1": W1, "W2": W2, "Wml": wml,
            "b1": b1[:, None].astype(np.float32),
            "b2": b2[:, None].astype(np.float32),
            "bml": bml,
            "iota": iota, "ident": ident,
        })

    from concourse.bass_utils import run_bass_kernel_spmd
    if trace:
        try:
            import types

            import antenv
            if "antenv.axon_hooks" not in sys.modules:
                mod = types.ModuleType("antenv.axon_hooks")
                mod._hook = None
                mod.set_axon_ntff_profile_hook = (
                    lambda h, _m=mod: setattr(_m, "_hook", h))
                mod.get_axon_ntff_profile_hook = lambda _m=mod: _m._hook
                sys.modules["antenv.axon_hooks"] = mod
                antenv.axon_hooks = mod
            hooks = sys.modules["antenv.axon_hooks"]
            if hooks.get_axon_ntff_profile_hook() is None:
                from trn_agent_boot.trn_boot import _ntff_profile_via_ctypes
                hooks.set_axon_ntff_profile_hook(
                    _ntff_profile_via_ctypes("/opt/axon/libaxon_pjrt.so"))
        except Exception as e:  # profiling is best-effort
            print(f"ntff hook registration failed: {e}")
    res = run_bass_kernel_spmd(nc, in_maps, list(range(cfg.n_cores)),
                               trace=trace)
    mu_pad = np.stack([res.results[c]["mu"] for c in range(cfg.n_cores)])
    lv_pad = np.stack([res.results[c]["lv"] for c in range(cfg.n_cores)])
    mu_pad = mu_pad.reshape(cfg.npad, out_c)
    lv_pad = lv_pad.reshape(cfg.npad, out_c)
    mu = mu_pad[gnode]
    lv = lv_pad[gnode]
    return (mu, lv), res


def kernel(**inputs):
    (mu, lv), _ = _run(inputs)
    return mu, lv


# revision 27
# speedup vs baseline: 1.0609x; 1.0609x over previous
"""GCN encoder (2x GCNConv + mu/logvar heads) on 8 Trainium2 NeuronCores.

Strategy (dst-sharded message passing, PE one-hot scatter):
  - Nodes are dst-sharded across 8 cores (6250 original rows each, padded
    to 6400 = 25 "pair-tiles" of 256 dst nodes).
  - All per-edge norm factors fold into the one-hot selection matrix built
    with a single fused DVE tensor_scalar (is_equal then *norm) per 128-edge
    subtile; scatter-add happens in PSUM via PE matmuls (float32r).
  - Source features are fetched with dma_gather (SDMA descriptor gather,
    512B rows) from a node table in DRAM. int16 gather indices force the
    table to be addressed as two halves (rows < 25600 and >= 25600).
  - The per-layer dense transform commutes with the scatter, so each layer
    is: aggregate (A^T tile in PSUM) -> W matmul -> bias+act -> transpose ->
    store node-major shard. mu/logvar share one aggregation pass.
  - Between layers, shards are exchanged with a shared-output AllGather into
    a Shared-scratchpad full table which the next pass gathers from.
"""

import os
import sys
from dataclasses import dataclass

import numpy as np

if "/opt/trn_rl_repo" not in sys.path:
    sys.path.insert(0, "/opt/trn_rl_repo")

N = 50000
E = 800000
IN_C = 128
HID = 128
OUT_C = 64
M = 8  # cores


@dataclass(frozen=True)
class Cfg:
    n: int            # real nodes
    n_cores: int
    shard: int        # real nodes per core (n = shard * n_cores)
    shard_pad: int    # padded nodes per core, multiple of 256
    table_split: int  # row count of the "lo" gather table (<= 32767)

    @property
    def npad(self):
        return self.shard_pad * self.n_cores

    @property
    def n_pair(self):
        return self.shard_pad // 256


REAL_CFG = Cfg(n=N, n_cores=M, shard=6250, shard_pad=6400, table_split=25600)

ST = 128  # edges per subtile
GSUB = int(os.environ.get("GSUB", "32"))  # max subtiles per dma_gather
SINGLE_PACKET = bool(int(os.environ.get("SINGLE_PACKET", "0")))
KNOCK_GATHER = bool(int(os.environ.get("KNOCK_GATHER", "0")))
KNOCK_SEL = bool(int(os.environ.get("KNOCK_SEL", "0")))
MM_SEG = int(os.environ.get("MM_SEG", "0"))  # 0 = one start/stop chain
# sel build engine: 0 = all DVE, 1 = alternate DVE/ACT, 2 = all ACT
SEL_ACT = int(os.environ.get("SEL_ACT", "1"))


def preprocess(x, edge_index, cfg: Cfg):
    """Sort/pad edges into the per-core static schedule + device input arrays."""
    x = np.asarray(x, np.float32)
    src = np.asarray(edge_index[0], np.int64)
    dst = np.asarray(edge_index[1], np.int64)
    n = cfg.n
    loop = np.arange(n, dtype=np.int64)
    src = np.concatenate([src, loop])
    dst = np.concatenate([dst, loop])
    deg = np.bincount(dst, minlength=n).astype(np.float32)
    dinv = 1.0 / np.sqrt(deg)
    norm = (dinv[src] * dinv[dst]).astype(np.float32)

    core = dst // cfg.shard
    dloc = dst % cfg.shard
    tile_id = dloc // 256
    dstloc = (dloc % 256).astype(np.float32)
    gsrc = (src // cfg.shard) * cfg.shard_pad + (src % cfg.shard)
    half = (gsrc >= cfg.table_split).astype(np.int64)
    gidx = np.where(half == 0, gsrc, gsrc - cfg.table_split).astype(np.int64)
    assert gidx.max() < 32768

    order = np.lexsort((gidx, half, tile_id, core))
    core, tile_id, half = core[order], tile_id[order], half[order]
    dstloc, norm, gidx = dstloc[order], norm[order], gidx[order]

    # counts[c, T, h]
    counts = np.zeros((cfg.n_cores, cfg.n_pair, 2), np.int64)
    np.add.at(counts, (core, tile_id, half), 1)
    nsub = np.ceil(counts.max(axis=0) / ST).astype(np.int64)  # [n_pair, 2]
    # every pair-tile contains real nodes -> self loops guarantee lo+hi >= 1;
    # still force at least one subtile total per tile for psum init
    for t in range(cfg.n_pair):
        if nsub[t].sum() == 0:
            nsub[t, 0] = 1
    S = int(nsub.sum())  # total subtiles per core

    # group start offsets (in subtiles), order: T0-lo, T0-hi, T1-lo, ...
    groups = []  # (tile, half, nsub, s0)
    s0 = 0
    for t in range(cfg.n_pair):
        for h in (0, 1):
            groups.append((t, h, int(nsub[t, h]), s0))
            s0 += int(nsub[t, h])

    # per-core padded edge arrays in schedule order
    idx_slab = np.zeros((cfg.n_cores, 128, S * 8), np.int16)
    dl_slab = np.zeros((cfg.n_cores, 128, S), np.float32)
    nm_slab = np.zeros((cfg.n_cores, 128, S), np.float32)

    # edges are sorted by (core, tile, half); find range per (c,t,h)
    key = (core * cfg.n_pair + tile_id) * 2 + half
    starts = np.searchsorted(key, np.arange(cfg.n_cores * cfg.n_pair * 2))
    ends = np.searchsorted(key, np.arange(cfg.n_cores * cfg.n_pair * 2) + 1)

    for c in range(cfg.n_cores):
        for (t, h, ns, g0) in groups:
            if ns == 0:
                continue
            k = (c * cfg.n_pair + t) * 2 + h
            a, b = starts[k], ends[k]
            cnt = b - a
            cap = ns * ST
            assert cnt <= cap
            gi = np.zeros(cap, np.int16)
            dl = np.zeros(cap, np.float32)
            nm = np.zeros(cap, np.float32)
            gi[:cnt] = gidx[a:b]
            dl[:cnt] = dstloc[a:b]
            nm[:cnt] = norm[a:b]
            cols = gi.reshape(ns * 8, 16).T  # [16, ns*8]
            idx_slab[c, :, g0 * 8:(g0 + ns) * 8] = np.tile(cols, (8, 1))
            dl_slab[c, :, g0:g0 + ns] = dl.reshape(ns, ST).T
            nm_slab[c, :, g0:g0 + ns] = nm.reshape(ns, ST).T

    # padded node table for x
    x_pad = np.zeros((cfg.npad, x.shape[1]), np.float32)
    gnode = (np.arange(n) // cfg.shard) * cfg.shard_pad + (np.arange(n) % cfg.shard)
    x_pad[gnode] = x

    return groups, S, idx_slab, dl_slab, nm_slab, x_pad, gnode


def build_program(cfg: Cfg, groups, S, feat_in, hid, out_c):
    from contextlib import ExitStack

    import concourse.tile as tile
    from concourse import bacc, mybir

    f32 = mybir.dt.float32
    f32r = mybir.dt.float32r
    i16 = mybir.dt.int16
    AF = mybir.ActivationFunctionType
    ALU = mybir.AluOpType

    nc = bacc.Bacc("TRN2", target_bir_lowering=False, debug=False,
                   num_devices=cfg.n_cores)

    npad, spad = cfg.npad, cfg.shard_pad
    xt = nc.dram_tensor("xt", [npad, feat_in], f32, kind="ExternalInput")
    idx_t = nc.dram_tensor("idx", [128, S * 8], i16, kind="ExternalInput")
    dl_t = nc.dram_tensor("dstloc", [128, S], f32, kind="ExternalInput")
    nm_t = nc.dram_tensor("norm", [128, S], f32, kind="ExternalInput")
    ndl_t = nc.dram_tensor("negdl", [128, S], f32, kind="ExternalInput")
    nnm_t = nc.dram_tensor("negnorm", [128, S], f32, kind="ExternalInput")
    w1_t = nc.dram_tensor("W1", [feat_in, hid], f32r, kind="ExternalInput")
    w2_t = nc.dram_tensor("W2", [hid, hid], f32r, kind="ExternalInput")
    wml_t = nc.dram_tensor("Wml", [hid, 2 * out_c], f32r, kind="ExternalInput")
    b1_t = nc.dram_tensor("b1", [hid, 1], f32, kind="ExternalInput")
    b2_t = nc.dram_tensor("b2", [hid, 1], f32, kind="ExternalInput")
    bml_t = nc.dram_tensor("bml", [2 * out_c, 1], f32, kind="ExternalInput")
    iota_t = nc.dram_tensor("iota", [128, 256], f32, kind="ExternalInput")
    ident_t = nc.dram_tensor("ident", [128, 128], f32, kind="ExternalInput")

    mu_t = nc.dram_tensor("mu", [spad, out_c], f32, kind="ExternalOutput")
    lv_t = nc.dram_tensor("lv", [spad, out_c], f32, kind="ExternalOutput")

    h1s = nc.dram_tensor("h1s", [spad, hid], f32)
    h2s = nc.dram_tensor("h2s", [spad, hid], f32)
    h1f = nc.dram_tensor("h1f", [npad, hid], f32, addr_space="Shared")
    h2f = nc.dram_tensor("h2f", [npad, hid], f32, addr_space="Shared")

    max_ns = max(g[2] for g in groups)
    replica = [list(range(cfg.n_cores))]

    with tile.TileContext(nc) as tc, ExitStack() as ctx:
        cpool = ctx.enter_context(tc.tile_pool(name="consts", bufs=1))
        gpool = ctx.enter_context(tc.tile_pool(name="gather", bufs=4))
        spool = ctx.enter_context(tc.tile_pool(name="sel", bufs=6))
        wpool = ctx.enter_context(tc.tile_pool(name="work", bufs=3))
        ppool = ctx.enter_context(tc.tile_pool(name="psum", bufs=2, space="PSUM"))
        ipool = ctx.enter_context(tc.tile_pool(name="iotap", bufs=1, space="PSUM"))

        def load_const(t, shape, dtype=f32):
            tl = cpool.tile(shape, dtype, tag=t.name)
            nc.sync.dma_start(tl[:], t.ap())
            return tl

        idx_sb = load_const(idx_t, [128, S * 8], i16)
        dl_sb = load_const(dl_t, [128, S])
        nm_sb = load_const(nm_t, [128, S])
        ndl_sb = load_const(ndl_t, [128, S])
        nnm_sb = load_const(nnm_t, [128, S])
        w1_sb = load_const(w1_t, [feat_in, hid], f32r)
        w2_sb = load_const(w2_t, [hid, hid], f32r)
        wml_sb = load_const(wml_t, [hid, 2 * out_c], f32r)
        b1_sb = load_const(b1_t, [hid, 1])
        b2_sb = load_const(b2_t, [hid, 1])
        bml_sb = load_const(bml_t, [2 * out_c, 1])
        iota_sb = load_const(iota_t, [128, 256])
        ident_sb = load_const(ident_t, [128, 128])
        # iota lives in PSUM: a DVE tensor_scalar with two SBUF tensor
        # operands enters 2-port perf mode, which locks GpSimd out of SBUF
        # while it is emitting SWDGE gather descriptors (hangs the device).
        iota_ps = ipool.tile([128, 256], f32, tag="iota_ps")
        nc.vector.tensor_copy(iota_ps[:], iota_sb[:])

        passes = [
            (xt, w1_sb, b1_sb, h1s, True),
            (h1f, w2_sb, b2_sb, h2s, True),
            (h2f, wml_sb, bml_sb, None, False),
        ]

        for pi, (table, w_sb, b_sb, shard_out, relu) in enumerate(passes):
            t_lo = table.ap()[:cfg.table_split, :]
            t_hi = table.ap()[cfg.table_split:, :]
            for t in range(cfg.n_pair):
                tgroups = [g for g in groups if g[0] == t and g[2] > 0]
                n_mm = sum(g[2] for g in tgroups)
                at_psum = ppool.tile([128, 256], f32, tag="at")
                mm = 0
                for (_, h, ns, s0) in tgroups:
                    src_ap = t_lo if h == 0 else t_hi
                    for c0 in range(0, ns, GSUB):
                        cn = min(GSUB, ns - c0)
                        g = gpool.tile([128, min(GSUB, max_ns), feat_in], f32r, tag="g")
                        if KNOCK_GATHER:
                            nc.vector.memset(g[:, :cn, :].bitcast(f32), 0.25)
                        else:
                            nc.gpsimd.dma_gather(
                                g[:, :cn, :], src_ap.bitcast(f32r),
                                idx_sb[:, (s0 + c0) * 8:(s0 + c0 + cn) * 8],
                                num_idxs=cn * ST, num_idxs_reg=cn * ST,
                                elem_size=feat_in,
                                single_packet=SINGLE_PACKET,
                            )
                        for k in range(cn):
                            s = s0 + c0 + k
                            sel = spool.tile([128, 256], f32r, tag="sel")
                            use_act = (SEL_ACT == 2) or (SEL_ACT == 1 and s % 2 == 1)
                            if KNOCK_SEL:
                                nc.vector.memset(sel[:].bitcast(f32), 0.001)
                            elif use_act:
                                # sel = relu(norm - norm*|iota - dstloc|) on the
                                # Scalar engine (per-partition bias/scale), which
                                # equals one-hot(dstloc)*norm for integer iota.
                                tmp = spool.tile([128, 256], f32, tag="seltmp")
                                nc.scalar.activation(
                                    tmp[:], iota_sb[:], AF.Abs,
                                    bias=ndl_sb[:, s:s + 1])
                                nc.scalar.activation(
                                    sel[:], tmp[:], AF.Relu,
                                    bias=nm_sb[:, s:s + 1],
                                    scale=nnm_sb[:, s:s + 1])
                            else:
                                nc.vector.tensor_scalar(
                                    sel[:], iota_ps[:],
                                    dl_sb[:, s:s + 1], nm_sb[:, s:s + 1],
                                    op0=ALU.is_equal, op1=ALU.mult,
                                )
                            if MM_SEG == 0:
                                start, stop = (mm == 0), (mm == n_mm - 1)
                            else:
                                start = mm == 0
                                stop = (mm % MM_SEG == MM_SEG - 1) or (mm == n_mm - 1)
                            nc.tensor.matmul(
                                at_psum[:],
                                lhsT=g[:, k, :],
                                rhs=sel[:],
                                start=start, stop=stop,
                            )
                            mm += 1
                at_sb = wpool.tile([128, 256], f32r, tag="at_sb")
                nc.vector.tensor_copy(at_sb[:], at_psum[:])
                ht_psum = ppool.tile([128, 256], f32, tag="ht")
                nc.tensor.matmul(ht_psum[:], lhsT=w_sb[:],
                                 rhs=at_sb[:],
                                 start=True, stop=True)
                h_sb = wpool.tile([128, 256], f32, tag="h_sb")
                nc.scalar.activation(h_sb[:], ht_psum[:],
                                     AF.Relu if relu else AF.Identity,
                                     bias=b_sb[:, :1])
                for j in (0, 1):
                    tp = ppool.tile([128, 128], f32, tag="tp")
                    nc.tensor.transpose(tp[:], h_sb[:, j * 128:(j + 1) * 128],
                                        ident_sb[:])
                    ts = wpool.tile([128, 128], f32, tag="ts")
                    nc.vector.tensor_copy(ts[:], tp[:])
                    r0 = t * 256 + j * 128
                    if shard_out is not None:
                        nc.sync.dma_start(out=shard_out.ap()[r0:r0 + 128, :],
                                          in_=ts[:])
                    else:
                        nc.sync.dma_start(out=mu_t.ap()[r0:r0 + 128, :],
                                          in_=ts[:, 0:out_c])
                        nc.sync.dma_start(out=lv_t.ap()[r0:r0 + 128, :],
                                          in_=ts[:, out_c:2 * out_c])
            if pi == 0:
                nc.gpsimd.collective_compute(
                    "AllGather", mybir.AluOpType.bypass,
                    replica_groups=replica,
                    ins=[h1s.ap()], outs=[h1f.ap()])
            elif pi == 1:
                nc.gpsimd.collective_compute(
                    "AllGather", mybir.AluOpType.bypass,
                    replica_groups=replica,
                    ins=[h2s.ap()], outs=[h2f.ap()])
    nc.compile()
    return nc


def _run(inputs, cfg=REAL_CFG, trace=False):
    x = np.asarray(inputs["x"], np.float32)
    edge_index = np.asarray(inputs["edge_index"])
    W1 = np.asarray(inputs["W1"], np.float32)
    b1 = np.asarray(inputs["b1"], np.float32)
    W2 = np.asarray(inputs["W2"], np.float32)
    b2 = np.asarray(inputs["b2"], np.float32)
    W_mu = np.asarray(inputs["W_mu"], np.float32)
    b_mu = np.asarray(inputs["b_mu"], np.float32)
    W_lv = np.asarray(inputs["W_lv"], np.float32)
    b_lv = np.asarray(inputs["b_lv"], np.float32)

    feat_in = x.shape[1]
    hid = W1.shape[1]
    out_c = W_mu.shape[1]

    groups, S, idx_slab, dl_slab, nm_slab, x_pad, gnode = preprocess(
        x, edge_index, cfg)
    nc = build_program(cfg, groups, S, feat_in, hid, out_c)

    wml = np.concatenate([W_mu, W_lv], axis=1)
    bml = np.concatenate([b_mu, b_lv])[:, None].astype(np.float32)
    iota = np.tile(np.arange(256, dtype=np.float32), (128, 1))
    ident = np.eye(128, dtype=np.float32)

    in_maps = []
    for c in range(cfg.n_cores):
        in_maps.append({
            "xt": x_pad,
            "idx": idx_slab[c],
            "dstloc": dl_slab[c],
            "norm": nm_slab[c],
            "negdl": -dl_slab[c],
            "negnorm": -nm_slab[c],
            "W1": W1, "W2": W2, "Wml": wml,
            "b1": b1[:, None].astype(np.float32),
            "b2": b2[:, None].astype(np.float32),
            "bml": bml,
            "iota": iota, "ident": ident,
        })

    from concourse.bass_utils import run_bass_kernel_spmd
    if trace:
        try:
            import types

            import antenv
            if "antenv.axon_hooks" not in sys.modules:
                mod = types.ModuleType("antenv.axon_hooks")
                mod._hook = None
                mod.set_axon_ntff_profile_hook = (
                    lambda h, _m=mod: setattr(_m, "_hook", h))
                mod.get_axon_ntff_profile_hook = lambda _m=mod: _m._hook
                sys.modules["antenv.axon_hooks"] = mod
                antenv.axon_hooks = mod
            hooks = sys.modules["antenv.axon_hooks"]
            if hooks.get_axon_ntff_profile_hook() is None:
                from trn_agent_boot.trn_boot import _ntff_profile_via_ctypes
                hooks.set_axon_ntff_profile_hook(
                    _ntff_profile_via_ctypes("/opt/axon/libaxon_pjrt.so"))
        except Exception as e:  # profiling is best-effort
            print(f"ntff hook registration failed: {e}")
    res = run_bass_kernel_spmd(nc, in_maps, list(range(cfg.n_cores)),
                               trace=trace)
    mu_pad = np.stack([res.results[c]["mu"] for c in range(cfg.n_cores)])
    lv_pad = np.stack([res.results[c]["lv"] for c in range(cfg.n_cores)])
    mu_pad = mu_pad.reshape(cfg.npad, out_c)
    lv_pad = lv_pad.reshape(cfg.npad, out_c)
    mu = mu_pad[gnode]
    lv = lv_pad[gnode]
    return (mu, lv), res


def kernel(**inputs):
    (mu, lv), _ = _run(inputs)
    return mu, lv
